# revision 35
# baseline (speedup 1.0000x reference)
# Multi-head attention on 8 Trainium2 NeuronCores.
#
# Sharding: 8 cores = 4 batches x 2 head-halves (tensor parallel). Each core
# computes QKV for its 6 heads over the full 2048-row batch, attention, and a
# partial output projection y_g = attn_g @ W_out[384g:384(g+1)]; the host sums
# the two partials per batch (free all-reduce). No K/V duplication.
#
# x/W/ident ship as bf16 (they were cast to bf16 on-chip anyway), halving
# input DMA and removing the weight-staging pass.
#
# Single fused pipeline, bf16 matmuls (fp8 was measured to give no PE speedup
# on hw: cost ~ moving columns streamed, independent of dtype):
#   prefix: W/x DMA, xT = transpose(x), qT/kT for head-pair 0, V rows 0..511
#   slots (h, qh, kt): score matmuls -> exp (ACT) -> pts ring; PV lagged 16
#   slots; V-proj and remaining QK-proj interleaved as fill work so the PE
#   stays dense while ACT streams exps.
#   Vn holds [V_h | 64 ones cols]: PV emits numerator rows 0:64 and the
#   denominator broadcast across rows 64:128; normalize = DVE rcp + mul.
# PSUM: sp 2x[128,1024] + pv 1x[128,1024] + transient 2x[128,512] = 8 banks.
import numpy as np

B, N, D = 4, 2048, 768
H, DH = 12, 64
HL = H // 2              # heads per core
DL = HL * DH             # 384 local qkv width
SCALE = DH ** -0.5
KT = D // 128            # 6 contraction tiles over D
CT = DL // 128           # 3 column tiles of local q/k
NKT = N // 128           # 16 key tiles
RT = N // 128            # 16 row tiles of x
NQH = 1024               # query-half processed per pv accumulation
LAG = 16                 # PV lags scores by one (h, qh) block
RING = 20                # pts ring tiles

_CACHE = {}


def _build(reps=1, variant="schrf"):
    key = ("nc", reps, variant)
    if key in _CACHE:
        return _CACHE[key]

    from concourse import bacc
    import concourse.tile as tile
    import concourse.mybir as mybir

    F32 = mybir.dt.float32
    BF16 = mybir.dt.bfloat16
    AF = mybir.ActivationFunctionType

    MMW = 512                 # score/PV matmul free width (ISA max 512 f32)
    # Slots whose exp runs on DVE/Pool as a Schraudolph bf16-bit trick
    # (out_bits = round(16256 + s * 128*SCALE/ln2), bitcast to bf16),
    # offloading softmax work from the Activation engine.
    DVE_KTS = ()
    if variant.startswith("schr"):
        DVE_KTS = (1, 4, 7, 10, 13, 15)
    A_SCHR = 128.0 * SCALE / float(np.log(2.0))
    B_SCHR = 16256.0 - 7.0  # zero-mean bias correction for the bits trick
    FRATE = 0.6 if variant.endswith("f") else 1.0

    nc = bacc.Bacc("TRN2", target_bir_lowering=False, debug=False,
                   num_devices=8)

    xT_d = nc.dram_tensor("x_t", [D, N], BF16, kind="ExternalInput").ap()
    wqkv = nc.dram_tensor("w_qkv", [D, 3 * DL], BF16,
                          kind="ExternalInput").ap()
    wout = nc.dram_tensor("w_out", [DL, D], BF16, kind="ExternalInput").ap()
    bout = nc.dram_tensor("b_out", [1, D], F32, kind="ExternalInput").ap()
    y = nc.dram_tensor("y", [N, D], F32, kind="ExternalOutput").ap()

    with tile.TileContext(nc) as tc:
      for _rep in range(reps):
        with tc.tile_pool(name="const", bufs=1) as const, \
             tc.tile_pool(name="persist", bufs=1) as persist, \
             tc.tile_pool(name="ptsp", bufs=RING) as ptsp, \
             tc.tile_pool(name="rstage", bufs=2) as rstage, \
             tc.tile_pool(name="ystage", bufs=2) as ystage, \
             tc.tile_pool(name="spp", bufs=2, space="PSUM") as spp, \
             tc.tile_pool(name="pvp", bufs=1, space="PSUM") as pvp, \
             tc.tile_pool(name="trp", bufs=2, space="PSUM") as trp:

            bias_bc = const.tile([128, D], F32)
            nc.gpsimd.dma_start(out=bias_bc, in_=bout.to_broadcast((128, D)))

            wqkv_bf = persist.tile([128, KT, 3 * DL], BF16)
            wout_bf = persist.tile([128, CT, D], BF16)
            xT = persist.tile([128, KT, N], BF16)
            qT = persist.tile([128, CT, N], BF16)
            kTt = persist.tile([128, CT, N], BF16)
            Vn = persist.tile([128, NKT, HL, 128], BF16)
            attnT = persist.tile([128, CT, N], BF16)

            nc.gpsimd.memset(Vn[:, :, :, DH:], 1.0)

            # ---- weight DMA on the Pool queue, q/k/v column groups in
            # need-order, so it streams in parallel with x on SP ----
            for c0 in (0, DL, 2 * DL):
                for j in range(KT):
                    nc.gpsimd.dma_start(
                        out=wqkv_bf[:, j, c0:c0 + DL],
                        in_=wqkv[j * 128:(j + 1) * 128, c0:c0 + DL])
            for j in range(CT):
                nc.gpsimd.dma_start(out=wout_bf[:, j, :],
                                    in_=wout[j * 128:(j + 1) * 128, :])

            # ---- x arrives pre-transposed; chunk so early rows unblock
            # the first projections quickly ----
            for rc in range(0, N, 512):
                for j in range(KT):
                    nc.sync.dma_start(
                        out=xT[:, j, rc:rc + 512],
                        in_=xT_d[j * 128:(j + 1) * 128, rc:rc + 512])

            # ---- fill jobs (emitted as micro-steps inside the slot loop) --
            def qk_steps(ct):
                # q and k interleaved by row chunk: scores for early key
                # tiles unblock as soon as the matching rows land
                for rc in range(0, N, 512):
                    yield ("qk", qT, ct, 0, rc)
                    yield ("qk", kTt, ct, DL, rc)

            def emit_qk(dst, ct, c0, rc, pool, tag):
                ps = pool.tile([128, 512], F32, tag=tag,
                               name=f"qk{c0}_{ct}_{rc}")
                cc = c0 + ct * 128
                for j in range(KT):
                    nc.tensor.matmul(ps, wqkv_bf[:, j, cc:cc + 128],
                                     xT[:, j, rc:rc + 512],
                                     start=(j == 0), stop=(j == KT - 1))
                nc.vector.tensor_copy(out=dst[:, ct, rc:rc + 512], in_=ps)

            def emit_v(rt, pool, tag):
                ps = pool.tile([128, DL], F32, tag=tag, name=f"v{rt}")
                for j in range(KT):
                    nc.tensor.matmul(
                        ps, xT[:, j, rt * 128:(rt + 1) * 128],
                        wqkv_bf[:, j, 2 * DL:3 * DL],
                        start=(j == 0), stop=(j == KT - 1))
                nc.vector.tensor_copy(
                    out=Vn[:, rt, :, 0:DH],
                    in_=ps.rearrange("p (h d) -> p h d", d=DH))

            # ---- prefix: qk ct0, V rt0-3 ----
            for step in qk_steps(0):
                emit_qk(step[1], step[2], step[3], step[4], trp, "tr")
            for rt in range(4):
                emit_v(rt, pvp, "pv")

            fill = []
            for rt in range(4, RT):
                fill.append(("v", rt))
            for ct in (1, 2):
                fill.extend(qk_steps(ct))
            fill_i = 0
            fill_budget = 0.0
            FILL_RATE = FRATE  # micro-steps per slot

            pts_tiles = [None] * (2 * HL * NKT)
            pv_tiles = {}

            def slot_decode(s):
                blk, kt = divmod(s, NKT)
                qh, h = divmod(blk, HL)
                return h, qh, kt

            def emit_proj(rt):
                ys = ystage.tile([128, D], F32, tag="ys", name=f"ys{rt}")
                for (c0, cw) in ((0, 512), (512, 256)):
                    yp = trp.tile([128, 512], F32, tag="tr", name=f"yp{rt}_{c0}")
                    for j in range(CT):
                        nc.tensor.matmul(
                            yp[:, 0:cw],
                            attnT[:, j, rt * 128:(rt + 1) * 128],
                            wout_bf[:, j, c0:c0 + cw],
                            start=(j == 0), stop=(j == CT - 1))
                    nc.vector.tensor_add(ys[:, c0:c0 + cw], yp[:, 0:cw],
                                         bias_bc[:, c0:c0 + cw])
                nc.sync.dma_start(out=y[rt * 128:(rt + 1) * 128, :], in_=ys)

            def emit_fill():
                nonlocal fill_i
                job = fill[fill_i]
                fill_i += 1
                if job[0] == "v":
                    emit_v(job[1], trp, "tr")
                elif job[0] == "proj":
                    emit_proj(job[1])
                else:
                    emit_qk(job[1], job[2], job[3], job[4], trp, "tr")

            def emit_pv(s):
                h, qh, kt = slot_decode(s)
                blk = h * 2 + qh
                if kt == 0:
                    pv_tiles[blk] = pvp.tile([128, NQH], F32, tag="pv",
                                             name=f"pv{blk}")
                pv = pv_tiles[blk]
                pt = pts_tiles[s]
                for rc in range(0, NQH, MMW):
                    nc.tensor.matmul(pv[:, rc:rc + MMW], Vn[:, kt, h, :],
                                     pt[:, rc:rc + MMW],
                                     start=(kt == 0), stop=(kt == NKT - 1))
                if kt == NKT - 1:
                    tj, po = divmod(h, 2)
                    po *= 64
                    rcp = rstage.tile([64, NQH], F32, tag="rcp",
                                      name=f"rcp{blk}")
                    nc.vector.reciprocal(rcp, pv[DH:DH + 64, :])
                    nc.vector.tensor_mul(
                        attnT[po:po + 64, tj, qh * NQH:(qh + 1) * NQH],
                        pv[0:DH, :], rcp)
                    del pv_tiles[blk]

            NOACT = variant == "noact"   # ablation: skip exp
            NOPV = variant == "nopv"     # ablation: skip PV/consume
            if NOPV:
                nc.vector.memset(attnT, 0.0)

            NSLOT = 2 * HL * NKT
            QH0_DONE = HL * NKT + LAG  # last qh0 consume emitted here
            for s in range(NSLOT + LAG):
                if s == QH0_DONE:
                    for rt in range(RT // 2):
                        fill.append(("proj", rt))
                fill_budget += FILL_RATE
                while fill_i < len(fill) and fill_budget >= 1.0:
                    emit_fill()
                    fill_budget -= 1.0
                if s >= LAG and not NOPV:
                    emit_pv(s - LAG)
                if s < NSLOT:
                    h, qh, kt = slot_decode(s)
                    tj, po = divmod(h, 2)
                    po *= 64
                    sp = spp.tile([128, NQH], F32, tag="sp", name=f"sp{s}")
                    for rc in range(0, NQH, MMW):
                        nc.tensor.matmul(
                            sp[:, rc:rc + MMW],
                            kTt[po:po + 64, tj, kt * 128:(kt + 1) * 128],
                            qT[po:po + 64, tj,
                               qh * NQH + rc:qh * NQH + rc + MMW],
                            start=True, stop=True)
                    pt = ptsp.tile([128, NQH], BF16, tag="pt", name=f"pt{s}")
                    if NOACT:
                        pass
                    elif kt in DVE_KTS:
                        nc.vector.tensor_scalar(
                            out=pt.bitcast(mybir.dt.int16), in0=sp,
                            scalar1=A_SCHR, scalar2=B_SCHR,
                            op0=mybir.AluOpType.mult,
                            op1=mybir.AluOpType.add)
                    else:
                        nc.scalar.activation(pt, sp, AF.Exp, 0.0, SCALE)
                    pts_tiles[s] = pt

            # ---- output projection for qh1 (qh0 streamed in-loop) ----
            for rt in range(RT // 2, RT):
                emit_proj(rt)

    nc.compile()
    _CACHE[key] = nc
    return nc


def _in_maps(x, W_qkv, W_out, b_out):
    import ml_dtypes
    BF = ml_dtypes.bfloat16
    x = np.asarray(np.asarray(x, dtype=np.float32), dtype=BF)
    W_qkv = np.asarray(np.asarray(W_qkv, dtype=np.float32), dtype=BF)
    W_out = np.asarray(np.asarray(W_out, dtype=np.float32), dtype=BF)
    b_out = np.ascontiguousarray(
        np.asarray(b_out, dtype=np.float32)).reshape(1, D)
    zeros_b = np.zeros((1, D), dtype=np.float32)
    xT = [np.ascontiguousarray(x[b].T) for b in range(B)]
    maps = []
    for c in range(8):
        b, g = divmod(c, 2)
        wq = np.ascontiguousarray(np.concatenate(
            [W_qkv[:, g * DL:(g + 1) * DL],
             W_qkv[:, D + g * DL:D + (g + 1) * DL],
             W_qkv[:, 2 * D + g * DL:2 * D + (g + 1) * DL]], axis=1))
        wo = np.ascontiguousarray(W_out[g * DL:(g + 1) * DL, :])
        maps.append({"x_t": xT[b], "w_qkv": wq, "w_out": wo,
                     "b_out": (b_out if g == 0 else zeros_b)})
    return maps


def kernel(x, W_qkv, W_out, b_out):
    from concourse import bass_utils
    nc = _build()
    maps = _in_maps(x, W_qkv, W_out, b_out)
    res = bass_utils.run_bass_kernel_spmd(nc, maps, core_ids=list(range(8)))
    out = np.empty((B, N, D), dtype=np.float32)
    for b in range(B):
        out[b] = res.results[2 * b]["y"] + res.results[2 * b + 1]["y"]
    return out
